# revision 7
# baseline (speedup 1.0000x reference)
"""Trainium2 Bass kernel for 3D neighborhood attention (sparse_attention).

Problem: q,k [1,40,40,40,48] fp32, rpb [8,3,3,3]; out [1,24,40,40,40].
Per voxel x: logits[h,kk] = scale * <q[x,h,:], k[x+off_kk,h,:]> + rpb[h,kk]
(zero-padded k at boundaries, kk over 3x3x3 offsets), p = softmax over kk,
out[x,h,:] = sum_kk p[h,kk] * off_kk  (constant integer offsets as values).

Sharding: spatial-parallel over H (40 -> 8 slabs of 5). Each core gets its
q slab plus a host-side im2col of the 27 shifted k views for its slab
(halo handled on host), so on-core everything is token-parallel with
tokens on SBUF partitions (2 tokens per partition) and no cross-partition
data movement. The PV contraction exploits that the "values" are the
constant offsets in {-1,0,1}^3: out_i = (sum of exp over di=+1 block) -
(sum over di=-1 block), so it is pure block reductions, no multiplies.
"""

import numpy as np

import concourse.bass as bass
import concourse.tile as tile
from concourse import bacc, mybir
from concourse.bass_utils import run_bass_kernel_spmd

NH = 8
HD = 6
DIM = NH * HD
KS = 3
NT = KS**3  # 27
SCALE = HD**-0.5
H = W = T = 40
N_CORES = 8
SLAB = H // N_CORES          # 5 rows of H per core
TOK = SLAB * W * T           # 8000 tokens per core
P = 128
TPP = 2                      # tokens per partition
TILES = 32                   # ceil(8000 / 256)
TOKP = TILES * P * TPP       # 8192
FKC = NT * DIM               # 1296  (kk, c) free dim per token
FKH = NT * NH                # 216   (kk, h) free dim per token

_prog_cache = {}


def _build_program():
    fp32 = mybir.dt.float32
    nc = bacc.Bacc("TRN2", target_bir_lowering=False, debug=False,
                   num_devices=N_CORES)
    qs = nc.dram_tensor("qs", [TILES, P, TPP * DIM], fp32,
                        kind="ExternalInput").ap()
    kn = nc.dram_tensor("kn", [TILES, P, TPP * FKC], fp32,
                        kind="ExternalInput").ap()
    rpbt = nc.dram_tensor("rpbt", [P, FKH], fp32, kind="ExternalInput").ap()
    out = nc.dram_tensor("out", [TILES, P, TPP * 3 * NH], fp32,
                         kind="ExternalOutput").ap()

    X = mybir.AxisListType.X
    XY = mybir.AxisListType.XY
    ADD = mybir.AluOpType.add

    with tile.TileContext(nc) as tc:
        with (
            tc.tile_pool(name="consts", bufs=1) as cpool,
            tc.tile_pool(name="kin", bufs=3) as kpool,
            tc.tile_pool(name="qin", bufs=3) as qpool,
            tc.tile_pool(name="prod", bufs=2) as ppool,
            tc.tile_pool(name="logit", bufs=3) as lpool,
            tc.tile_pool(name="expv", bufs=3) as epool,
            tc.tile_pool(name="small", bufs=16) as spool,
            tc.tile_pool(name="outp", bufs=3) as opool,
        ):
            rpb_sb = cpool.tile([P, FKH], fp32)
            nc.sync.dma_start(rpb_sb[:], rpbt[:])

            for ti in range(TILES):
                kt = kpool.tile([P, TPP * FKC], fp32)
                nc.sync.dma_start(kt[:], kn[ti])
                qt = qpool.tile([P, TPP * DIM], fp32)
                nc.sync.dma_start(qt[:], qs[ti])

                # P4[p, j, kk, c] = kn[p, j, kk, c] * q[p, j, c]
                pt = ppool.tile([P, TPP * FKC], fp32)
                q_b = (qt[:].rearrange("p (j c) -> p j c", j=TPP)
                       .unsqueeze(2).broadcast_to([P, TPP, NT, DIM]))
                nc.vector.tensor_mul(
                    pt[:].rearrange("p (j kk c) -> p j kk c", j=TPP, kk=NT),
                    kt[:].rearrange("p (j kk c) -> p j kk c", j=TPP, kk=NT),
                    q_b,
                )
                # L[p, (j,kk,h)] = sum_d P4[p, j, (kk,h), d]
                lt = lpool.tile([P, TPP * FKH], fp32)
                nc.vector.tensor_reduce(
                    lt[:],
                    pt[:].rearrange("p (j kh d) -> p j kh d", j=TPP, d=HD),
                    axis=X, op=ADD,
                )
                # L2 = L + rpb  (q was pre-scaled by SCALE on host)
                l2 = lpool.tile([P, TPP * FKH], fp32)
                rpb_b = rpb_sb[:].unsqueeze(1).broadcast_to([P, TPP, FKH])
                nc.vector.tensor_add(
                    l2[:].rearrange("p (j f) -> p j f", j=TPP),
                    lt[:].rearrange("p (j f) -> p j f", j=TPP),
                    rpb_b,
                )
                # E = exp(L2)  (ScalarE, overlaps with DVE)
                et = epool.tile([P, TPP * FKH], fp32)
                nc.scalar.activation(et[:], l2[:],
                                     mybir.ActivationFunctionType.Exp)

                # Softmax denominator: S0[p, (j,h)] = sum_kk E
                e_khk = et[:].rearrange("p (j kk h) -> p j h kk",
                                        j=TPP, kk=NT, h=NH)
                s0 = spool.tile([P, TPP * NH], fp32)
                nc.vector.tensor_reduce(s0[:], e_khk, axis=X, op=ADD)

                # Directional numerators via paired block sums over the
                # +-1 slabs of each axis (values are +-1/0).
                # E free layout: (j, di, dj, dl, h).  V layout: (o, j, pm, h)
                v_di = et[:].rearrange(
                    "p (j di dj dl h) -> p j di h (dj dl)",
                    j=TPP, di=KS, dj=KS, dl=KS, h=NH)
                v_dj = et[:].rearrange(
                    "p (j di dj dl h) -> p j dj h di dl",
                    j=TPP, di=KS, dj=KS, dl=KS, h=NH)
                v_dl = et[:].rearrange(
                    "p (j di dj dl h) -> p j dl h di dj",
                    j=TPP, di=KS, dj=KS, dl=KS, h=NH)

                vt = spool.tile([P, 3 * 2 * TPP * NH], fp32)  # [128, 96]
                npm = TPP * NH
                for o, (v, ax) in enumerate(((v_di, X), (v_dj, XY),
                                             (v_dl, XY))):
                    for pm in range(2):
                        nc.vector.tensor_reduce(
                            vt[:, (o * 2 + pm) * npm:(o * 2 + pm + 1) * npm],
                            v[:, :, 2 * pm], axis=ax, op=ADD)

                # S3[p, (o,j,h)] = V[.., pm=1] - V[.., pm=0]
                v5 = vt[:].rearrange("p (o pm j h) -> p o pm j h",
                                     o=3, pm=2, j=TPP)
                s3 = spool.tile([P, 3 * TPP * NH], fp32)
                nc.vector.tensor_sub(
                    s3[:].rearrange("p (o j h) -> p o j h", o=3, j=TPP),
                    v5[:, :, 1], v5[:, :, 0])

                rt = spool.tile([P, TPP * NH], fp32)
                nc.vector.reciprocal(rt[:], s0[:])
                # out[p, (o,j,h)] = S3 * (1/S0)
                ot = opool.tile([P, TPP * 3 * NH], fp32)
                r_b = (rt[:].rearrange("p (j h) -> p j h", j=TPP)
                       .unsqueeze(1).broadcast_to([P, 3, TPP, NH]))
                nc.vector.tensor_mul(
                    ot[:].rearrange("p (o j h) -> p o j h", o=3, j=TPP),
                    s3[:].rearrange("p (o j h) -> p o j h", o=3, j=TPP),
                    r_b)
                nc.sync.dma_start(out[ti], ot[:])

    nc.compile()
    return nc


def _host_prep(q, k, rpb):
    q = np.asarray(q, dtype=np.float32)
    k = np.asarray(k, dtype=np.float32)
    rpb = np.asarray(rpb, dtype=np.float32)

    q0 = (q[0] * SCALE).astype(np.float32)          # [40,40,40,48]
    kp = np.pad(k[0], ((1, 1), (1, 1), (1, 1), (0, 0)))  # [42,42,42,48]
    win = np.lib.stride_tricks.sliding_window_view(kp, (KS, KS, KS),
                                                   axis=(0, 1, 2))
    # win: [40,40,40,48,3,3,3] -> [40,40,40,(kk,c)]
    win = np.ascontiguousarray(win.transpose(0, 1, 2, 4, 5, 6, 3))
    win = win.reshape(H, W, T, FKC)

    rpb_kh = np.ascontiguousarray(rpb.reshape(NH, NT).T).reshape(FKH)
    rpb_t = np.broadcast_to(rpb_kh, (P, FKH)).copy()

    in_maps = []
    for i in range(N_CORES):
        h0 = i * SLAB
        q_pad = np.zeros((TOKP, DIM), np.float32)
        q_pad[:TOK] = q0[h0:h0 + SLAB].reshape(TOK, DIM)
        kn_pad = np.zeros((TOKP, FKC), np.float32)
        kn_pad[:TOK] = win[h0:h0 + SLAB].reshape(TOK, FKC)
        in_maps.append({
            "qs": q_pad.reshape(TILES, P, TPP * DIM),
            "kn": kn_pad.reshape(TILES, P, TPP * FKC),
            "rpbt": rpb_t,
        })
    return in_maps


def _assemble(results):
    slabs = []
    for i in range(N_CORES):
        o = results[i]["out"].reshape(TILES, P, 3, TPP, NH)
        o = o.transpose(0, 1, 3, 2, 4).reshape(TOKP, 3, NH)[:TOK]
        o = o.reshape(SLAB, W, T, 3, NH)
        # channel order in reference: c = h*3 + o
        slabs.append(o.transpose(0, 1, 2, 4, 3).reshape(SLAB, W, T, 3 * NH))
    full = np.concatenate(slabs, axis=0)             # [40,40,40,24]
    return np.ascontiguousarray(full.transpose(3, 0, 1, 2))[None]


def _run(q, k, rpb, **spmd_kwargs):
    if "prog" not in _prog_cache:
        _prog_cache["prog"] = _build_program()
    nc = _prog_cache["prog"]
    in_maps = _host_prep(q, k, rpb)
    res = run_bass_kernel_spmd(nc, in_maps, list(range(N_CORES)),
                               **spmd_kwargs)
    return _assemble(res.results), res


def kernel(q, k, rpb):
    out, _ = _run(q, k, rpb)
    return out
